# revision 12
# baseline (speedup 1.0000x reference)
"""Trainium2 Bass kernel for nn_Controller (dense_mlp).

Key structural fact about the reference model: the Controller evaluates 16
tiny MLPs over the whole batch, but the rule/strength computation reads only
batch element 0 (`m[0]`), producing 2 scalars that are softmaxed and
broadcast to every row.  So the mathematically-exact computation is:

    m[k]      = sigmoid( relu(relu(s[0, k%4]*W1[k]+b1[k]) @ W2[k] + b2[k]) @ W3[k] + b3[k] )
    delta     = mean(m[0:8]) - mean(m[8:16])
    out[r, :] = [sigmoid(delta), sigmoid(-delta)]   for every row r
                (softmax of 2 logits == sigmoid of their difference)

Each of the 8 cores runs the identical tiny MLP (replicated weights, ~17K
MACs) and writes its 1/8 slice of the (262144, 2) output — data-parallel over
the batch dimension, with the scalar strengths recomputed on every device
instead of broadcast.

Device layout: the 16 nets x 8 output-groups are spread over all 128 SBUF
partitions: partition p handles net k = p//8, outputs o in {4g..4g+3} with
g = p%8.  The per-net layer-1 activations are computed redundantly in every
group so layer 2 becomes a fully partition-parallel fragment product; the
8 per-net partial dot products of layer 3 are summed across partitions with
a one-hot selector matmul on the PE.  b2 is folded into the layer-2
reduction as a 33rd i-slot against a constant-1.0 activation column.

All device inputs are packed into one (128, 220) f32 tensor so a single DMA
brings everything on-chip.  Column layout per partition p (k=p//8, g=p%8):

    col 0        : x[k]     = s[0, k % 4]
    cols 1:33    : W1[k,0,:]                          (32)
    cols 33:65   : b1[k,:]                            (32)
    cols 65:197  : [W2[k, :, 4g+j], b2[k, 4g+j]] j=0..3  (4*33, j-major)
    cols 197:201 : W3[k, 4g+j, 0]                     (4)
    cols 201:217 : one-hot selector S[p, :] = e_k     (16)
    cols 217:219 : G2 rows (partitions 0..15 only) = [+/-0.125, -/+0.125]
    col 219      : b3[p, 0] (partitions 0..15 only)

Raw Bass (no Tile): the whole kernel is one serial dependency chain, so
semaphores are placed by hand.  Two hardware facts drive the structure:
compute-engine pipelines do NOT enforce same-engine RAW hazards (explicit
drain() between dependent ops), and a plain .then_inc can fire before the
writes land (cross-engine signals use maybe_drain_then_inc).  No nc.Block()
is used: engines simply halt when done, which skips the end-of-kernel
all-engine barrier; the final sync-engine wait on the output-DMA semaphore
keeps the NEFF alive until the result is in HBM.
"""

import numpy as np

N_CORES = 8
B = 262144
ROWS_PER_CORE = B // N_CORES      # 32768
P = 128                           # output staging partitions
FREE = ROWS_PER_CORE * 2 // P     # 512 f32 per partition = 2KB

# packed column offsets
C_XK = 0
C_W1 = 1
C_B1 = 33
C_W2 = 65                         # 4 fragments x 33 (32 W2 + 1 b2)
C_W3 = C_W2 + 4 * 33              # 197
C_SEL = C_W3 + 4                  # 201
C_G2 = C_SEL + 16                 # 217
C_B3 = C_G2 + 2                   # 219
C_TOT = 220

_CACHE = {}


def _build_bass():
    import concourse.bass as bass
    from concourse import mybir

    f32 = mybir.dt.float32
    Alu = mybir.AluOpType
    Act = mybir.ActivationFunctionType

    nc = bass.Bass()
    packed_d = nc.dram_tensor("packed", [P, C_TOT], f32, kind="ExternalInput")
    out_d = nc.dram_tensor("out", [P, FREE], f32, kind="ExternalOutput")

    with (
        nc.sbuf_tensor([P, C_TOT], f32) as pk,
        nc.sbuf_tensor([P, 33], f32) as t1,
        nc.sbuf_tensor([P, 4, 33], f32) as prod,
        nc.sbuf_tensor([P, 4], f32) as h2p,
        nc.sbuf_tensor([P, 4], f32) as junk,
        nc.sbuf_tensor([P, 1], f32) as partials,
        nc.sbuf_tensor([16, P], f32) as mwide,
        nc.sbuf_tensor([P, FREE], f32) as outt,
        nc.psum_tensor([16, 1], f32) as psA,
        nc.psum_tensor([P, 2], f32) as ps2,
        nc.semaphore("dsem") as dsem,
        nc.semaphore("vsem") as vsem,
        nc.semaphore("ssem") as ssem,
        nc.semaphore("tsem") as tsem,
        nc.semaphore("osem") as osem,
    ):
        xk = pk[:, C_XK:C_W1]                                    # (128,1)
        W1r = pk[:, C_W1:C_B1]                                   # (128,32)
        b1r = pk[:, C_B1:C_W2]                                   # (128,32)
        W2f = pk[:, C_W2:C_W3].rearrange("p (j i) -> p j i", i=33)
        W3f = pk[:, C_W3:C_SEL]                                  # (128,4)
        Sel = pk[:, C_SEL:C_G2]                                  # (128,16)
        G2 = pk[0:16, C_G2:C_B3]                                 # (16,2)
        b3 = pk[0:16, C_B3:C_TOT]                                # (16,1)

        # --- DMA (sync sequencer / HWDGE ring) ---
        nc.sync.dma_start(out=pk[:, :], in_=packed_d[:, :]).then_inc(dsem, 16)

        # --- DVE: the 16 tiny MLPs, partition-parallel ---
        # constant-1.0 slot for the b2 fold, written before the DMA wait
        nc.vector.memset(t1[:, 32:33], 1.0)
        nc.vector.wait_ge(dsem, 16)
        # layer 1 pre-relu: t1 = x * W1 + b1      (each net x8 groups)
        nc.vector.scalar_tensor_tensor(
            out=t1[:, 0:32], in0=W1r, scalar=xk, in1=b1r,
            op0=Alu.mult, op1=Alu.add)
        nc.vector.drain()
        # layer 2 fragment product, relu fused, b2 rides along on the
        # constant-1.0 slot: prod = relu([t1, 1.0]) * [W2f, b2]
        nc.vector.scalar_tensor_tensor(
            out=prod[:, :, :],
            in0=t1[:, 0:33].unsqueeze(1).broadcast_to((P, 4, 33)),
            scalar=0.0, in1=W2f,
            op0=Alu.max, op1=Alu.mult)
        nc.vector.drain()
        nc.vector.reduce_sum(
            out=h2p[:, :], in_=prod[:, :, :], axis=mybir.AxisListType.X)
        nc.vector.drain()
        # layer 3 partial dots, relu fused:
        # partials[p] = sum_j relu(h2p[p,j]) * W3f[p,j]
        nc.vector.scalar_tensor_tensor(
            out=junk[:, :], in0=h2p[:, :], scalar=0.0, in1=W3f,
            op0=Alu.max, op1=Alu.mult, accum_out=partials[:, :])
        nc.vector.maybe_drain_then_inc((vsem, 1))

        # --- PE: cross-partition sums + delta broadcast ---
        # (vsem>=1 transitively implies the input DMA completed)
        nc.tensor.wait_ge(vsem, 1)
        # per-net sums across the 8 groups: psA[k] = sum_p Sel[p,k]*partials[p]
        nc.tensor.matmul(psA[:, :], Sel, partials[:, :], start=True, stop=True)
        nc.tensor.maybe_drain_then_inc((tsem, 1))
        nc.tensor.wait_ge(ssem, 1)
        # ps2[p, a] = sum_k m[k] * G2[k, a] = [delta, -delta] on all
        # 128 partitions in one matmul.
        nc.tensor.matmul(ps2[:, :], mwide[:, :], G2, start=True, stop=True)
        nc.tensor.maybe_drain_then_inc((tsem, 2))

        # --- ACT: sigmoids + output pattern fill ---
        nc.scalar.wait_ge(tsem, 1)
        # m = sigmoid(psA + b3), replicated 128x along free dim to serve
        # as the matmul stationary operand with M=128.
        nc.scalar.activation(
            out=mwide[:, :], in_=psA[:, :].to_broadcast((16, P)),
            func=Act.Sigmoid, bias=b3, scale=1.0)
        nc.scalar.maybe_drain_then_inc((ssem, 1))
        nc.scalar.wait_ge(tsem, 2)
        # softmax probs + pattern fill: out[p, r, :] = sigmoid([delta,-delta])
        nc.scalar.activation(
            out=outt[:, :].rearrange("p (r c) -> p r c", c=2),
            in_=ps2[:, :].unsqueeze(1).broadcast_to((P, FREE // 2, 2)),
            func=Act.Sigmoid)
        nc.scalar.maybe_drain_then_inc((ssem, 2))

        # --- store + completion hold ---
        nc.sync.wait_ge(ssem, 2)
        nc.sync.dma_start(out=out_d[:, :], in_=outt[:, :]).then_inc(osem, 16)
        nc.sync.wait_ge(osem, 16)

    return nc


def _pack_inputs(s, W1, b1, W2, b2, W3, b3):
    s = np.asarray(s, np.float32)
    W1 = np.asarray(W1, np.float32)
    b1 = np.asarray(b1, np.float32)
    W2 = np.asarray(W2, np.float32)
    b2 = np.asarray(b2, np.float32)
    W3 = np.asarray(W3, np.float32)
    b3 = np.asarray(b3, np.float32)

    k = np.arange(P) // 8                  # net handled by partition p
    g = np.arange(P) % 8                   # output group handled by p
    o = (g[:, None] * 4 + np.arange(4)[None, :])   # (128, 4) output cols

    packed = np.zeros((P, C_TOT), np.float32)
    packed[:, C_XK] = s[0, k % 4]
    packed[:, C_W1:C_B1] = W1[k, 0, :]
    packed[:, C_B1:C_W2] = b1[k, :]
    # fragment j: 32 W2 values then the matching b2 value (b2 fold)
    w2fb = np.empty((P, 4, 33), np.float32)
    w2fb[:, :, :32] = W2[k[:, None, None], :, o[:, :, None]][:, :, 0, :]
    w2fb[:, :, 32] = b2[k[:, None], o]
    packed[:, C_W2:C_W3] = w2fb.reshape(P, 132)
    packed[:, C_W3:C_SEL] = W3[k[:, None], o, 0]
    packed[np.arange(P), C_SEL + k] = 1.0
    sgn = np.where(np.arange(16) < 8, 0.125, -0.125).astype(np.float32)
    packed[0:16, C_G2] = sgn
    packed[0:16, C_G2 + 1] = -sgn
    packed[0:16, C_B3] = b3[:, 0]
    return packed


def kernel(s, W1, b1, W2, b2, W3, b3, _trace=False):
    from concourse.bass_utils import run_bass_kernel_spmd

    if "nc" not in _CACHE:
        _CACHE["nc"] = _build_bass()
    nc = _CACHE["nc"]

    packed = _pack_inputs(s, W1, b1, W2, b2, W3, b3)
    in_maps = [{"packed": packed} for _ in range(N_CORES)]
    res = run_bass_kernel_spmd(
        nc, in_maps, core_ids=list(range(N_CORES)), trace=_trace)
    out = np.concatenate(
        [r["out"].reshape(ROWS_PER_CORE, 2) for r in res.results], axis=0)
    if _trace:
        _CACHE["last_result"] = res
    return out
